# revision 9
# baseline (speedup 1.0000x reference)
"""Trainium2 Bass kernel for nn_CrossAttentionAdapter.

Math note: the reference's attention has kv_len == 1, so the softmax over a
length-1 axis is exactly 1.0 and the attention output is just `v` broadcast
over the P=32 prefix positions.  The whole module therefore collapses to a
chain of 4 matmuls applied to image_embs:

    row = image_embs @ Wm.T @ Wv.T @ Wo_mha.T @ Wo.T  (+ bias constant)
    out[b, p, :] = row[b, :]          for every p in range(32)

where Wv = Win[2E:3E].  The bias terms contribute a batch-independent
constant vector c = ((bm @ Wv.T + bv) @ Wo_mha.T + bo_mha) @ Wo.T + bo,
added on the host.  prefix_queries / Wq / Wk never affect the output.

Device strategy (pure data parallel, 8 cores), v3:
  - batch (1024) sharded 8-ways -> 128 rows per core
  - ALL four weights are quantized on the host to int8 with
    per-input-channel (per-k) scales; activations are fp16.  This halves
    the weight HBM traffic (28MB -> 14MB per core), which was the
    baseline bottleneck (~76us DMA busy at 102us total).
  - int8 chunks are upcast to fp16 on-chip: a plain dtype-cast copy,
    split DVE (2 elem/cycle/lane) / ACT; the dequant scales are NOT
    applied here -- per-k scales commute through the matmul onto the
    activations, so layer l's input scales are applied for free in layer
    l-1's PSUM-evacuation copy (activation Copy with a per-partition
    scale vector), and layer 1's scales are folded into xT on the host.
  - all 4 layers run weight-stationary (weight tile lhsT [k,m], acts
    moving N=128): LDWEIGHTS is ~fully hidden at this shape (61ns/MM
    measured).  Layer outputs stay feature-on-partitions so the chain
    needs no transposes; the host untransposes the final (feat, batch)
    tiles during unshard.
  - 16 output-tile accumulators per layer packed 4-per-PSUM-bank as
    [128,512] tiles; bank-interleaved m order for ILP across banks.
  - chunk plans: small leading chunks (L1: 1,1,2,2,2 slabs) so the first
    matmul starts ~9us, and small trailing chunks (L4: ...,2,2) to cut
    the DMA->dequant->PE tail latency.

walrus in this environment accepts only ONE semaphore wait per
instruction; `_legalize_waits` splits any extra waits into standalone
single-wait NoOps spliced immediately before the instruction on the same
engine stream (FIFO dispatch makes this exactly equivalent).
"""

import os
from contextlib import ExitStack

import numpy as np

import concourse.bass as bass
import concourse.mybir as mybir
import concourse.tile as tile
from concourse.bass import _add_dep_helper
from concourse.bass_utils import run_bass_kernel_spmd

B, CLIP, P, E, H = 1024, 1024, 32, 2048, 16
NCORES = 8
BC = B // NCORES  # batch rows per core

CHUNK_COLS = 4 * E          # largest chunk: 4 k-slabs
DVE_FRAC = 47               # DVE's share of dequant cols, out of 64


def _dve_cols(total):
    return ((DVE_FRAC * total) // 64) // 128 * 128


# per-layer k-slab chunk plans (sum == n_k_slabs)
PLANS = [
    [1, 1, 2, 2, 2],        # L1: 8 slabs,  fast start
    [4, 4, 4, 4],           # L2
    [4, 4, 4, 4],           # L3
    [4, 4, 4, 2, 2],        # L4: short tail
]


def _build_kernel(tc, out_ap, xT, wq_aps, s_aps):
    nc = tc.nc
    f32 = mybir.dt.float32
    f16 = mybir.dt.float16
    i8 = mybir.dt.int8
    COPY = mybir.ActivationFunctionType.Copy

    with ExitStack() as ctx:
        const_pool = ctx.enter_context(tc.tile_pool(name="const", bufs=1))
        i8pool = ctx.enter_context(tc.tile_pool(name="i8chunk", bufs=1))
        wpool = ctx.enter_context(tc.tile_pool(name="wchunk", bufs=1))
        act_pool = ctx.enter_context(tc.tile_pool(name="act", bufs=8))
        out_pool = ctx.enter_context(tc.tile_pool(name="out", bufs=1))
        acc_pool = ctx.enter_context(
            tc.tile_pool(name="acc", bufs=7, space=bass.MemorySpace.PSUM)
        )
        warm_pool = ctx.enter_context(
            tc.tile_pool(name="warm", bufs=1, space=bass.MemorySpace.PSUM)
        )

        # xT: (CLIP, BC) -> 8 stacked [128, 128] tiles in one DMA on the SP
        # HWDGE queue so the Pool queue starts streaming weights immediately
        x_t = const_pool.tile([128, (CLIP // 128) * BC], f16, name="xT_sb", tag="xT")
        nc.sync.dma_start(x_t[:], xT.rearrange("(t p) c -> p t c", p=128))
        actT = [x_t[:, bass.ts(k, BC)] for k in range(CLIP // 128)]

        # per-layer input-scale tiles for layers 2..4: [128, 16] fp32,
        # column mt = scales for features mt*128+p of the previous output
        s_sb = []
        for li, s_ap in enumerate(s_aps):
            st = const_pool.tile([128, 16], f32, name=f"s{li+2}_sb", tag=f"s{li+2}")
            nc.sync.dma_start(st[:], s_ap)
            s_sb.append(st)

        NI8 = 6
        i8tiles = [
            i8pool.tile([128, CHUNK_COLS], i8, name=f"i8buf{i}", tag=f"i8buf{i}")
            for i in range(NI8)
        ]
        NWBUF = 5
        wtiles = [
            wpool.tile([128, CHUNK_COLS], f16, name=f"wbuf{i}", tag=f"wbuf{i}")
            for i in range(NWBUF)
        ]

        # warm-up: the PE idles ~8us waiting for the first weight chunk and
        # would start HAM-throttled (1.2GHz for ~3.4us).  Issue dummy matmuls
        # on a scratch tile during that window so real matmuls start warm.
        wu = const_pool.tile([128, 128], f16, name="warm", tag="warm")
        wu_ps = warm_pool.tile([128, 128], f32, name="warm_ps", tag="warm_ps")
        nc.vector.memset(wu[:], 0.0)
        for _ in range(56):
            nc.tensor.matmul(wu_ps[:], wu[:], wu[:], start=True, stop=True)

        # bank-interleaved m order so consecutive matmuls hit different
        # PSUM banks (instruction-level parallelism across banks)
        m_order = [c + 4 * g for g in range(4) for c in range(4)]

        i8_count = 0
        w_count = 0
        for li, wT in enumerate(wq_aps):
            plan = PLANS[li]
            nk = sum(plan)
            last = li == len(wq_aps) - 1
            # slab-major stride view: [128, nk, E]
            wT_v = wT.rearrange("(s p) c -> p s c", p=128)
            accs = [
                acc_pool.tile([128, 512], f32, name="acc", tag="acc")
                for _ in range(4)
            ]
            bank_start_mm = {}
            if last:
                out_sb = out_pool.tile([128, E], f32, name="out_sb", tag="out_sb")
                groups = None
            else:
                out_sb = None
                groups = [
                    act_pool.tile([128, 512], f16, name="actg", tag="actg")
                    for _ in range(4)
                ]
                snext = s_sb[li]
            s0 = 0
            for T in plan:
                cols = T * E
                dcols = _dve_cols(cols)
                ichunk = i8tiles[i8_count % NI8]
                i8_count += 1
                nc.gpsimd.dma_start(ichunk[:, :cols], wT_v[:, s0 : s0 + T])
                wchunk = wtiles[w_count % NWBUF]
                w_count += 1
                # dequant: plain dtype cast, split DVE / ACT
                nc.vector.tensor_copy(wchunk[:, :dcols], ichunk[:, :dcols])
                nc.scalar.copy(wchunk[:, dcols:cols], ichunk[:, dcols:cols])
                for t in range(T):
                    k = s0 + t
                    fin = k == nk - 1
                    # on the final k-slab go bank-major so each bank's
                    # evacuation can start while other banks still accumulate
                    order = list(range(16)) if fin else m_order
                    for m in order:
                        sl, bank = m % 4, m // 4
                        # start=True clears has_written for the WHOLE bank,
                        # so only the first slice written into each bank may
                        # set it; later slices' first matmuls overwrite via
                        # the cleared bits (and must be ordered after the
                        # clearing matmul).
                        mm = nc.tensor.matmul(
                            accs[bank][:, sl * 128 : (sl + 1) * 128],
                            wchunk[:, t * E + m * 128 : t * E + (m + 1) * 128],
                            actT[k],
                            start=(k == 0 and sl == 0),
                            stop=(fin and sl == 3),
                            skip_group_check=True,
                        )
                        if k == 0:
                            if sl == 0:
                                bank_start_mm[bank] = mm
                            else:
                                _add_dep_helper(
                                    mm.ins, bank_start_mm[bank].ins,
                                    sync=False, reason="bank clear order",
                                )
                        if fin and sl == 3:
                            if last:
                                # plain fp32 evacuation of m-tiles 4b..4b+3,
                                # then store while other banks still run
                                nc.scalar.copy(
                                    out_sb[:, bank * 512 : (bank + 1) * 512],
                                    accs[bank][:],
                                )
                                (nc.sync if bank % 2 == 0 else nc.gpsimd).dma_start(
                                    out_ap[:, bass.ts(bank, 512)],
                                    out_sb[:, bass.ts(bank, 512)],
                                )
                            else:
                                # evacuate with the NEXT layer's per-k input
                                # scales (per-partition scale vector)
                                for sl2 in range(4):
                                    mt = bank * 4 + sl2
                                    nc.scalar.activation(
                                        groups[bank][:, sl2 * 128 : (sl2 + 1) * 128],
                                        accs[bank][:, sl2 * 128 : (sl2 + 1) * 128],
                                        COPY,
                                        scale=snext[:, mt : mt + 1],
                                    )
                s0 += T
            if not last:
                actT = [
                    groups[k // 4][:, (k % 4) * 128 : (k % 4 + 1) * 128]
                    for k in range(E // 128)
                ]


def _legalize_waits(nc):
    """walrus here accepts only one semaphore wait per instruction.  Split
    any extra waits into standalone single-wait NoOps spliced immediately
    before the instruction on the same engine stream; engine dispatch is
    strictly FIFO, so the semantics are identical."""
    wid = [0]
    for f in nc.m.functions:
        for blk in f.blocks:
            insts = list(blk.instructions)
            new = []
            changed = False
            for inst in insts:
                si = getattr(inst, "sync_info", None)
                w = list(si.on_wait) if si is not None and si.on_wait else []
                if len(w) > 1:
                    changed = True
                    for x in w[:-1]:
                        nop = mybir.InstNoOp(
                            name=f"Wsplit-{wid[0]}", ins=[], outs=[]
                        )
                        wid[0] += 1
                        nop.engine = inst.engine
                        nop.sync_info = mybir.SyncInfo(
                            on_wait=[x], on_update=[]
                        )
                        new.append(nop)
                    upd = list(si.on_update) if si.on_update else []
                    inst.sync_info = mybir.SyncInfo(on_wait=[w[-1:][0]], on_update=upd)
                new.append(inst)
            if changed:
                blk.instructions = new


_NC_CACHE = None


def _get_nc(legalize=True):
    global _NC_CACHE
    if legalize and _NC_CACHE is not None:
        return _NC_CACHE
    nc = bass.Bass("TRN2", target_bir_lowering=False, debug=False)
    f16 = mybir.dt.float16
    i8 = mybir.dt.int8
    f32 = mybir.dt.float32
    xT = nc.dram_tensor("xT", (CLIP, BC), f16, kind="ExternalInput")
    wmq = nc.dram_tensor("wmq", (CLIP, E), i8, kind="ExternalInput")
    wvq = nc.dram_tensor("wvq", (E, E), i8, kind="ExternalInput")
    womq = nc.dram_tensor("womq", (E, E), i8, kind="ExternalInput")
    woq = nc.dram_tensor("woq", (E, E), i8, kind="ExternalInput")
    s2 = nc.dram_tensor("s2", (128, 16), f32, kind="ExternalInput")
    s3 = nc.dram_tensor("s3", (128, 16), f32, kind="ExternalInput")
    s4 = nc.dram_tensor("s4", (128, 16), f32, kind="ExternalInput")
    # out is the TRANSPOSED row block: out[p, mt*128 + b] = y[mt*128+p, b]
    out = nc.dram_tensor("out", (128, E), f32, kind="ExternalOutput")
    with tile.TileContext(nc) as tc:
        _build_kernel(
            tc,
            out.ap(),
            xT.ap(),
            [wmq.ap(), wvq.ap(), womq.ap(), woq.ap()],
            [s2.ap(), s3.ap(), s4.ap()],
        )
    if not legalize:
        return nc
    _legalize_waits(nc)
    _NC_CACHE = nc
    return nc


LAST_RESULTS = None  # BassKernelResults of the most recent run (for profiling)


def _ensure_ntff_hook():
    """Register the axon NTFF profiling hook if the image's antenv lacks it."""
    try:
        from antenv.axon_hooks import get_axon_ntff_profile_hook  # noqa: F401

        return
    except ImportError:
        pass
    import sys as _sys
    import types as _types

    try:
        from trn_agent_boot.trn_boot import _ntff_profile_via_ctypes

        hook = _ntff_profile_via_ctypes("/opt/axon/libaxon_pjrt.so")
    except Exception:
        hook = None
    mod = _types.ModuleType("antenv.axon_hooks")
    mod._hook = hook
    mod.get_axon_ntff_profile_hook = lambda: mod._hook
    mod.set_axon_ntff_profile_hook = lambda h: setattr(mod, "_hook", h)
    _sys.modules["antenv.axon_hooks"] = mod
    import antenv

    antenv.axon_hooks = mod
    # artifact upload needs S3 egress which this sandbox doesn't have
    import concourse.bass_utils as _bu

    _bu.upload_artifacts = lambda tmpdir: tmpdir


def _quant_per_k(W):
    """Per-input-channel int8 quantization of W.T: returns (Q (K,M) int8,
    s (K,) fp32) with W.T ~= s[:,None] * Q."""
    wT = np.ascontiguousarray(W.T).astype(np.float32)
    s = np.abs(wT).max(axis=1) / 127.0
    s = np.where(s == 0, 1.0, s)
    Q = np.rint(wT / s[:, None]).astype(np.int8)
    return Q, s.astype(np.float32)


def kernel(image_embs, Wm, bm, prefix_queries, Win, bin, Wo_mha, bo_mha, Wo, bo):
    X = np.asarray(image_embs, dtype=np.float32)
    Wm = np.asarray(Wm, dtype=np.float32)
    bm = np.asarray(bm, dtype=np.float32)
    Win = np.asarray(Win, dtype=np.float32)
    bin_ = np.asarray(bin, dtype=np.float32)
    Wo_mha = np.asarray(Wo_mha, dtype=np.float32)
    bo_mha = np.asarray(bo_mha, dtype=np.float32)
    Wo = np.asarray(Wo, dtype=np.float32)
    bo = np.asarray(bo, dtype=np.float32)

    Wv = Win[2 * E : 3 * E]
    bv = bin_[2 * E : 3 * E]

    # batch-independent bias contribution (exact, fp32 on host)
    c = ((bm @ Wv.T + bv) @ Wo_mha.T + bo_mha) @ Wo.T + bo  # (E,)

    qm, sm = _quant_per_k(Wm)        # L1 input scales -> folded into xT
    qv, sv = _quant_per_k(Wv)        # L2 input scales -> applied at L1 evac
    qom, som = _quant_per_k(Wo_mha)  # L3 -> L2 evac
    qo, so = _quant_per_k(Wo)        # L4 -> L3 evac

    shared = {
        "wmq": qm,
        "wvq": qv,
        "womq": qom,
        "woq": qo,
        "s2": np.ascontiguousarray(sv.reshape(16, 128).T),
        "s3": np.ascontiguousarray(som.reshape(16, 128).T),
        "s4": np.ascontiguousarray(so.reshape(16, 128).T),
    }
    in_maps = []
    for ci in range(NCORES):
        xs = X[ci * BC : (ci + 1) * BC]  # (BC, CLIP)
        m = dict(shared)
        # xT carries L1's per-k dequant scales
        m["xT"] = np.ascontiguousarray(xs.T * sm[:, None]).astype(np.float16)
        in_maps.append(m)

    nc = _get_nc()
    trace = bool(int(os.environ.get("KERNEL_TRACE", "0")))
    if trace:
        _ensure_ntff_hook()
    res = run_bass_kernel_spmd(
        nc, in_maps, core_ids=list(range(NCORES)), trace=trace
    )
    global LAST_RESULTS
    LAST_RESULTS = res

    # out[p, mt*128+b] = y[mt*128+p, b]; untranspose per 128-col tile
    rows = np.empty((B, E), np.float32)
    for ci in range(NCORES):
        o = np.asarray(res.results[ci]["out"]).reshape(128, 16, BC)
        rows[ci * BC : (ci + 1) * BC] = o.transpose(2, 1, 0).reshape(BC, E)
    rows = rows + c[None, :].astype(np.float32)
    return np.broadcast_to(rows[:, None, :], (B, P, E))


# revision 10
# speedup vs baseline: 1.0640x; 1.0640x over previous
"""Trainium2 Bass kernel for nn_CrossAttentionAdapter.

Math note: the reference's attention has kv_len == 1, so the softmax over a
length-1 axis is exactly 1.0 and the attention output is just `v` broadcast
over the P=32 prefix positions.  The whole module therefore collapses to a
chain of 4 matmuls applied to image_embs:

    row = image_embs @ Wm.T @ Wv.T @ Wo_mha.T @ Wo.T  (+ bias constant)
    out[b, p, :] = row[b, :]          for every p in range(32)

where Wv = Win[2E:3E].  The bias terms contribute a batch-independent
constant vector c = ((bm @ Wv.T + bv) @ Wo_mha.T + bo_mha) @ Wo.T + bo,
added on the host.  prefix_queries / Wq / Wk never affect the output.

Device strategy (pure data parallel, 8 cores), v3:
  - batch (1024) sharded 8-ways -> 128 rows per core
  - ALL four weights are quantized on the host to int8 with
    per-input-channel (per-k) scales; activations are fp16.  This halves
    the weight HBM traffic (28MB -> 14MB per core), which was the
    baseline bottleneck (~76us DMA busy at 102us total).
  - int8 chunks are upcast to fp16 on-chip: a plain dtype-cast copy,
    split DVE (2 elem/cycle/lane) / ACT; the dequant scales are NOT
    applied here -- per-k scales commute through the matmul onto the
    activations, so layer l's input scales are applied for free in layer
    l-1's PSUM-evacuation copy (activation Copy with a per-partition
    scale vector), and layer 1's scales are folded into xT on the host.
  - all 4 layers run weight-stationary (weight tile lhsT [k,m], acts
    moving N=128): LDWEIGHTS is ~fully hidden at this shape (61ns/MM
    measured).  Layer outputs stay feature-on-partitions so the chain
    needs no transposes; the host untransposes the final (feat, batch)
    tiles during unshard.
  - 16 output-tile accumulators per layer packed 4-per-PSUM-bank as
    [128,512] tiles; bank-interleaved m order for ILP across banks.
  - chunk plans: small leading chunks (L1: 1,1,2,2,2 slabs) so the first
    matmul starts ~9us, and small trailing chunks (L4: ...,2,2) to cut
    the DMA->dequant->PE tail latency.

walrus in this environment accepts only ONE semaphore wait per
instruction; `_legalize_waits` splits any extra waits into standalone
single-wait NoOps spliced immediately before the instruction on the same
engine stream (FIFO dispatch makes this exactly equivalent).
"""

import os
from contextlib import ExitStack

import numpy as np

import concourse.bass as bass
import concourse.mybir as mybir
import concourse.tile as tile
from concourse.bass import _add_dep_helper
from concourse.bass_utils import run_bass_kernel_spmd

B, CLIP, P, E, H = 1024, 1024, 32, 2048, 16
NCORES = 8
BC = B // NCORES  # batch rows per core

CHUNK_COLS = 4 * E          # largest chunk: 4 k-slabs
DVE_FRAC = 47               # DVE's share of dequant cols, out of 64


def _dve_cols(total):
    return ((DVE_FRAC * total) // 64) // 128 * 128


# per-layer k-slab chunk plans (sum == n_k_slabs)
PLANS = [
    [1, 1, 2, 2, 2],        # L1: 8 slabs,  fast start
    [4, 4, 4, 4],           # L2
    [4, 4, 4, 4],           # L3
    [4, 4, 4, 2, 2],        # L4: short tail
]


def _build_kernel(tc, out_ap, xT, wq_aps, s_aps):
    nc = tc.nc
    f32 = mybir.dt.float32
    f16 = mybir.dt.float16
    i8 = mybir.dt.int8
    COPY = mybir.ActivationFunctionType.Copy

    with ExitStack() as ctx:
        const_pool = ctx.enter_context(tc.tile_pool(name="const", bufs=1))
        i8pool = ctx.enter_context(tc.tile_pool(name="i8chunk", bufs=1))
        wpool = ctx.enter_context(tc.tile_pool(name="wchunk", bufs=1))
        act_pool = ctx.enter_context(tc.tile_pool(name="act", bufs=8))
        out_pool = ctx.enter_context(tc.tile_pool(name="out", bufs=1))
        acc_pool = ctx.enter_context(
            tc.tile_pool(name="acc", bufs=8, space=bass.MemorySpace.PSUM)
        )

        # xT: (CLIP, BC) -> 8 stacked [128, 128] tiles in one DMA on the SP
        # HWDGE queue so the Pool queue starts streaming weights immediately
        x_t = const_pool.tile([128, (CLIP // 128) * BC], f16, name="xT_sb", tag="xT")
        nc.sync.dma_start(x_t[:], xT.rearrange("(t p) c -> p t c", p=128))
        actT = [x_t[:, bass.ts(k, BC)] for k in range(CLIP // 128)]

        # per-layer input-scale tiles for layers 2..4: [128, 16] fp32,
        # column mt = scales for features mt*128+p of the previous output
        s_sb = []
        for li, s_ap in enumerate(s_aps):
            st = const_pool.tile([128, 16], f32, name=f"s{li+2}_sb", tag=f"s{li+2}")
            nc.sync.dma_start(st[:], s_ap)
            s_sb.append(st)

        NI8 = 6
        i8tiles = [
            i8pool.tile([128, CHUNK_COLS], i8, name=f"i8buf{i}", tag=f"i8buf{i}")
            for i in range(NI8)
        ]
        NWBUF = 5
        wtiles = [
            wpool.tile([128, CHUNK_COLS], f16, name=f"wbuf{i}", tag=f"wbuf{i}")
            for i in range(NWBUF)
        ]

        # bank-interleaved m order so consecutive matmuls hit different
        # PSUM banks (instruction-level parallelism across banks)
        m_order = [c + 4 * g for g in range(4) for c in range(4)]

        i8_count = 0
        w_count = 0
        for li, wT in enumerate(wq_aps):
            plan = PLANS[li]
            nk = sum(plan)
            last = li == len(wq_aps) - 1
            # slab-major stride view: [128, nk, E]
            wT_v = wT.rearrange("(s p) c -> p s c", p=128)
            accs = [
                acc_pool.tile([128, 512], f32, name="acc", tag="acc")
                for _ in range(4)
            ]
            bank_start_mm = {}
            if last:
                out_sb = out_pool.tile([128, E], f32, name="out_sb", tag="out_sb")
                groups = None
            else:
                out_sb = None
                groups = [
                    act_pool.tile([128, 512], f16, name="actg", tag="actg")
                    for _ in range(4)
                ]
                snext = s_sb[li]
            s0 = 0
            for T in plan:
                cols = T * E
                dcols = _dve_cols(cols)
                ichunk = i8tiles[i8_count % NI8]
                i8_count += 1
                nc.gpsimd.dma_start(ichunk[:, :cols], wT_v[:, s0 : s0 + T])
                wchunk = wtiles[w_count % NWBUF]
                w_count += 1
                # dequant: plain dtype cast, split DVE / ACT
                nc.vector.tensor_copy(wchunk[:, :dcols], ichunk[:, :dcols])
                nc.scalar.copy(wchunk[:, dcols:cols], ichunk[:, dcols:cols])
                for t in range(T):
                    k = s0 + t
                    fin = k == nk - 1
                    # on the final k-slab go bank-major so each bank's
                    # evacuation can start while other banks still accumulate
                    order = list(range(16)) if fin else m_order
                    for m in order:
                        sl, bank = m % 4, m // 4
                        # start=True clears has_written for the WHOLE bank,
                        # so only the first slice written into each bank may
                        # set it; later slices' first matmuls overwrite via
                        # the cleared bits (and must be ordered after the
                        # clearing matmul).
                        mm = nc.tensor.matmul(
                            accs[bank][:, sl * 128 : (sl + 1) * 128],
                            wchunk[:, t * E + m * 128 : t * E + (m + 1) * 128],
                            actT[k],
                            start=(k == 0 and sl == 0),
                            stop=(fin and sl == 3),
                            skip_group_check=True,
                        )
                        if k == 0:
                            if sl == 0:
                                bank_start_mm[bank] = mm
                            else:
                                _add_dep_helper(
                                    mm.ins, bank_start_mm[bank].ins,
                                    sync=False, reason="bank clear order",
                                )
                        if fin and sl == 3:
                            if last:
                                # plain fp32 evacuation of m-tiles 4b..4b+3,
                                # then store while other banks still run
                                nc.scalar.copy(
                                    out_sb[:, bank * 512 : (bank + 1) * 512],
                                    accs[bank][:],
                                )
                                (nc.sync if bank % 2 == 0 else nc.gpsimd).dma_start(
                                    out_ap[:, bass.ts(bank, 512)],
                                    out_sb[:, bass.ts(bank, 512)],
                                )
                            else:
                                # evacuate with the NEXT layer's per-k input
                                # scales (per-partition scale vector)
                                for sl2 in range(4):
                                    mt = bank * 4 + sl2
                                    nc.scalar.activation(
                                        groups[bank][:, sl2 * 128 : (sl2 + 1) * 128],
                                        accs[bank][:, sl2 * 128 : (sl2 + 1) * 128],
                                        COPY,
                                        scale=snext[:, mt : mt + 1],
                                    )
                s0 += T
            if not last:
                actT = [
                    groups[k // 4][:, (k % 4) * 128 : (k % 4 + 1) * 128]
                    for k in range(E // 128)
                ]


def _legalize_waits(nc):
    """walrus here accepts only one semaphore wait per instruction.  Split
    any extra waits into standalone single-wait NoOps spliced immediately
    before the instruction on the same engine stream; engine dispatch is
    strictly FIFO, so the semantics are identical."""
    wid = [0]
    for f in nc.m.functions:
        for blk in f.blocks:
            insts = list(blk.instructions)
            new = []
            changed = False
            for inst in insts:
                si = getattr(inst, "sync_info", None)
                w = list(si.on_wait) if si is not None and si.on_wait else []
                if len(w) > 1:
                    changed = True
                    for x in w[:-1]:
                        nop = mybir.InstNoOp(
                            name=f"Wsplit-{wid[0]}", ins=[], outs=[]
                        )
                        wid[0] += 1
                        nop.engine = inst.engine
                        nop.sync_info = mybir.SyncInfo(
                            on_wait=[x], on_update=[]
                        )
                        new.append(nop)
                    upd = list(si.on_update) if si.on_update else []
                    inst.sync_info = mybir.SyncInfo(on_wait=[w[-1:][0]], on_update=upd)
                new.append(inst)
            if changed:
                blk.instructions = new


_NC_CACHE = None


def _get_nc(legalize=True):
    global _NC_CACHE
    if legalize and _NC_CACHE is not None:
        return _NC_CACHE
    nc = bass.Bass("TRN2", target_bir_lowering=False, debug=False)
    f16 = mybir.dt.float16
    i8 = mybir.dt.int8
    f32 = mybir.dt.float32
    xT = nc.dram_tensor("xT", (CLIP, BC), f16, kind="ExternalInput")
    wmq = nc.dram_tensor("wmq", (CLIP, E), i8, kind="ExternalInput")
    wvq = nc.dram_tensor("wvq", (E, E), i8, kind="ExternalInput")
    womq = nc.dram_tensor("womq", (E, E), i8, kind="ExternalInput")
    woq = nc.dram_tensor("woq", (E, E), i8, kind="ExternalInput")
    s2 = nc.dram_tensor("s2", (128, 16), f32, kind="ExternalInput")
    s3 = nc.dram_tensor("s3", (128, 16), f32, kind="ExternalInput")
    s4 = nc.dram_tensor("s4", (128, 16), f32, kind="ExternalInput")
    # out is the TRANSPOSED row block: out[p, mt*128 + b] = y[mt*128+p, b]
    out = nc.dram_tensor("out", (128, E), f32, kind="ExternalOutput")
    with tile.TileContext(nc) as tc:
        _build_kernel(
            tc,
            out.ap(),
            xT.ap(),
            [wmq.ap(), wvq.ap(), womq.ap(), woq.ap()],
            [s2.ap(), s3.ap(), s4.ap()],
        )
    if not legalize:
        return nc
    _legalize_waits(nc)
    _NC_CACHE = nc
    return nc


LAST_RESULTS = None  # BassKernelResults of the most recent run (for profiling)


def _ensure_ntff_hook():
    """Register the axon NTFF profiling hook if the image's antenv lacks it."""
    try:
        from antenv.axon_hooks import get_axon_ntff_profile_hook  # noqa: F401

        return
    except ImportError:
        pass
    import sys as _sys
    import types as _types

    try:
        from trn_agent_boot.trn_boot import _ntff_profile_via_ctypes

        hook = _ntff_profile_via_ctypes("/opt/axon/libaxon_pjrt.so")
    except Exception:
        hook = None
    mod = _types.ModuleType("antenv.axon_hooks")
    mod._hook = hook
    mod.get_axon_ntff_profile_hook = lambda: mod._hook
    mod.set_axon_ntff_profile_hook = lambda h: setattr(mod, "_hook", h)
    _sys.modules["antenv.axon_hooks"] = mod
    import antenv

    antenv.axon_hooks = mod
    # artifact upload needs S3 egress which this sandbox doesn't have
    import concourse.bass_utils as _bu

    _bu.upload_artifacts = lambda tmpdir: tmpdir


def _quant_per_k(W):
    """Per-input-channel int8 quantization of W.T: returns (Q (K,M) int8,
    s (K,) fp32) with W.T ~= s[:,None] * Q."""
    wT = np.ascontiguousarray(W.T).astype(np.float32)
    s = np.abs(wT).max(axis=1) / 127.0
    s = np.where(s == 0, 1.0, s)
    Q = np.rint(wT / s[:, None]).astype(np.int8)
    return Q, s.astype(np.float32)


def kernel(image_embs, Wm, bm, prefix_queries, Win, bin, Wo_mha, bo_mha, Wo, bo):
    X = np.asarray(image_embs, dtype=np.float32)
    Wm = np.asarray(Wm, dtype=np.float32)
    bm = np.asarray(bm, dtype=np.float32)
    Win = np.asarray(Win, dtype=np.float32)
    bin_ = np.asarray(bin, dtype=np.float32)
    Wo_mha = np.asarray(Wo_mha, dtype=np.float32)
    bo_mha = np.asarray(bo_mha, dtype=np.float32)
    Wo = np.asarray(Wo, dtype=np.float32)
    bo = np.asarray(bo, dtype=np.float32)

    Wv = Win[2 * E : 3 * E]
    bv = bin_[2 * E : 3 * E]

    # batch-independent bias contribution (exact, fp32 on host)
    c = ((bm @ Wv.T + bv) @ Wo_mha.T + bo_mha) @ Wo.T + bo  # (E,)

    qm, sm = _quant_per_k(Wm)        # L1 input scales -> folded into xT
    qv, sv = _quant_per_k(Wv)        # L2 input scales -> applied at L1 evac
    qom, som = _quant_per_k(Wo_mha)  # L3 -> L2 evac
    qo, so = _quant_per_k(Wo)        # L4 -> L3 evac

    shared = {
        "wmq": qm,
        "wvq": qv,
        "womq": qom,
        "woq": qo,
        "s2": np.ascontiguousarray(sv.reshape(16, 128).T),
        "s3": np.ascontiguousarray(som.reshape(16, 128).T),
        "s4": np.ascontiguousarray(so.reshape(16, 128).T),
    }
    in_maps = []
    for ci in range(NCORES):
        xs = X[ci * BC : (ci + 1) * BC]  # (BC, CLIP)
        m = dict(shared)
        # xT carries L1's per-k dequant scales
        m["xT"] = np.ascontiguousarray(xs.T * sm[:, None]).astype(np.float16)
        in_maps.append(m)

    nc = _get_nc()
    trace = bool(int(os.environ.get("KERNEL_TRACE", "0")))
    if trace:
        _ensure_ntff_hook()
    res = run_bass_kernel_spmd(
        nc, in_maps, core_ids=list(range(NCORES)), trace=trace
    )
    global LAST_RESULTS
    LAST_RESULTS = res

    # out[p, mt*128+b] = y[mt*128+p, b]; untranspose per 128-col tile
    rows = np.empty((B, E), np.float32)
    for ci in range(NCORES):
        o = np.asarray(res.results[ci]["out"]).reshape(128, 16, BC)
        rows[ci * BC : (ci + 1) * BC] = o.transpose(2, 1, 0).reshape(BC, E)
    rows = rows + c[None, :].astype(np.float32)
    return np.broadcast_to(rows[:, None, :], (B, P, E))
